# revision 5
# baseline (speedup 1.0000x reference)
"""Trainium2 Bass kernel for nn_Classifier0 (quadrant-sum classifier).

reference:
    agg[n, q]  = quadrant sums of x[n] (512x512, quadrants of 256x256)
    w          = g * v[..., 0] / ||v||            [4, 4]
    y          = agg[:, :, None] * w + b_fgl      [N, 4, 4]
    out        = y.reshape(N, 16) @ W_fc.T + b_fc [N, 10]

Algebraic refactor (exact in real arithmetic):
    out[n, c] = sum_q agg[n, q] * A[q, c] + cc[c]
      A[q, c] = sum_j w[q, j] * W_fc[c, 4q + j]         (4 x 10, host, fp64)
      cc[c]   = b_fgl.ravel() @ W_fc[c] + b_fc[c]       (10, host, fp64)

Device work (data-parallel, 32 samples per core, C samples per DMA chunk):
    - per chunk k: DMA a [128, FREE] tile; partition p holds RPP = 512/PPS
      consecutive image rows of sample C*k + p//PPS  (PPS = 128/C).
      DVE tensor_reduce sums the left 256 columns, ACT (activation Copy
      with accum_out) sums the right 256 -> bufL/bufR [128, NCH].
    - p % PPS < PPS/2 is the image's top half, so quadrant sums are a
      single matmul contraction over all 128 partitions with zero-masked
      weights (mask isolates each sample subgroup j):
        psum[k, j*10+c] = sum_p bufL[p,k] walm[p, j*10+c]
                        + sum_p bufR[p,k] warm[p, j*10+c] + cc[c]
      psum [NCH, C*10] row-major IS y [32, 10] with n = C*k + j; one copy
      to SBUF and one contiguous DMA writes the final output.
"""

import numpy as np

N, S = 256, 512
H = S // 2
NCORES = 8
SPC = N // NCORES  # samples per core (32)
NCLS = 10

C = 4  # samples per DMA chunk
NCH = SPC // C  # chunks per core
PPS = 128 // C  # partitions per sample
RPP = S // PPS  # image rows per partition
FREE = S * RPP  # floats per partition per chunk

_PROGRAM_CACHE = {}


def _build_program():
    from contextlib import ExitStack

    import concourse.bacc as bacc
    import concourse.mybir as mybir
    import concourse.tile as tile

    nc = bacc.Bacc("TRN2", target_bir_lowering=False, debug=False)
    dt = mybir.dt.float32

    x_t = nc.dram_tensor("x", [NCH, 128, FREE], dt, kind="ExternalInput")
    wal_t = nc.dram_tensor("walm", [128, C * NCLS], dt, kind="ExternalInput")
    war_t = nc.dram_tensor("warm", [128, C * NCLS], dt, kind="ExternalInput")
    ccb_t = nc.dram_tensor("ccbt", [1, C * NCLS], dt, kind="ExternalInput")
    y_t = nc.dram_tensor("y", [SPC, NCLS], dt, kind="ExternalOutput")

    with tile.TileContext(nc) as tc, ExitStack() as ctx:
        xpool = ctx.enter_context(tc.tile_pool(name="xp", bufs=3))
        spool = ctx.enter_context(tc.tile_pool(name="sp", bufs=2))
        cpool = ctx.enter_context(tc.tile_pool(name="cp", bufs=1))
        ppool = ctx.enter_context(tc.tile_pool(name="pp", bufs=1, space="PSUM"))

        x_ap = x_t.ap()

        bufL = cpool.tile([128, NCH], dt)
        bufR = cpool.tile([128, NCH], dt)
        # constant loads go on the idle GpSimd SWDGE queue so the SP queue
        # starts streaming x immediately
        walm = cpool.tile([128, C * NCLS], dt)
        nc.gpsimd.dma_start(walm[:], wal_t.ap())
        warm = cpool.tile([128, C * NCLS], dt)
        nc.gpsimd.dma_start(warm[:], war_t.ap())
        ccbt = cpool.tile([1, C * NCLS], dt)
        nc.gpsimd.dma_start(ccbt[:], ccb_t.ap())
        ones1 = cpool.tile([1, NCH], dt)
        nc.vector.memset(ones1[:], 1.0)

        for k in range(NCH):
            xt = xpool.tile([128, FREE], dt)
            nc.sync.dma_start(xt[:], x_ap[k])
            xv = xt[:].rearrange("p (r c) -> p r c", c=S)
            # left 256 columns of each of the RPP rows in this partition
            nc.vector.tensor_reduce(
                bufL[:, k : k + 1],
                xv[:, :, 0:H],
                axis=mybir.AxisListType.XY,
                op=mybir.AluOpType.add,
            )
            # right half on the scalar engine via activation accumulate
            scratch = spool.tile([128, RPP * H], dt)
            sv = scratch[:].rearrange("p (r c) -> p r c", c=H)
            nc.scalar.activation(
                sv,
                xv[:, :, H:S],
                mybir.ActivationFunctionType.Copy,
                accum_out=bufR[:, k : k + 1],
            )

        psum = ppool.tile([NCH, C * NCLS], dt)
        nc.tensor.matmul(psum[:], lhsT=bufL[:], rhs=walm[:], start=True, stop=False)
        nc.tensor.matmul(psum[:], lhsT=bufR[:], rhs=warm[:], start=False, stop=False)
        nc.tensor.matmul(psum[:], lhsT=ones1[:], rhs=ccbt[:], start=False, stop=True)

        out_sb = cpool.tile([NCH, C * NCLS], dt)
        nc.vector.tensor_copy(out_sb[:], psum[:])
        nc.sync.dma_start(y_t.ap().rearrange("(k j) c -> k (j c)", j=C), out_sb[:])

    nc.compile()
    return nc


def _host_params(v, g, b_fgl, W_fc, b_fc):
    """Fold the tiny params into masked walm/warm [128, C*10] and cc [1, C*10]."""
    v64 = v.astype(np.float64)
    w = g.astype(np.float64) * (v64[..., 0] / np.linalg.norm(v64, axis=-1))  # [4,4]
    A = np.einsum("qj,cqj->qc", w, W_fc.astype(np.float64).reshape(NCLS, 4, 4))
    cc = b_fgl.astype(np.float64).reshape(-1) @ W_fc.astype(np.float64).T
    cc = cc + b_fc.astype(np.float64)

    # quadrant ids: 0=TL, 1=BL, 2=BR, 3=TR; p % PPS < PPS/2 -> top half rows
    p = np.arange(128)
    top = (p % PPS) < (PPS // 2)
    al_col = np.where(top[:, None], A[0][None, :], A[1][None, :])  # [128, 10]
    ar_col = np.where(top[:, None], A[3][None, :], A[2][None, :])  # [128, 10]
    # mask: partition p belongs to sample subgroup j = p // PPS
    grp = p // PPS  # [128]
    walm = np.zeros((128, C * NCLS), np.float64)
    warm = np.zeros((128, C * NCLS), np.float64)
    for j in range(C):
        sel = grp == j
        walm[sel, j * NCLS : (j + 1) * NCLS] = al_col[sel]
        warm[sel, j * NCLS : (j + 1) * NCLS] = ar_col[sel]
    ccbt = np.tile(cc, C).reshape(1, C * NCLS)
    return (
        np.ascontiguousarray(walm, dtype=np.float32),
        np.ascontiguousarray(warm, dtype=np.float32),
        np.ascontiguousarray(ccbt, dtype=np.float32),
    )


def _run(inputs, trace=False):
    from concourse.bass_utils import run_bass_kernel_spmd

    if "nc" not in _PROGRAM_CACHE:
        _PROGRAM_CACHE["nc"] = _build_program()
    nc = _PROGRAM_CACHE["nc"]

    x = np.asarray(inputs["x"], dtype=np.float32)
    walm, warm, ccbt = _host_params(
        np.asarray(inputs["v"], np.float32),
        np.asarray(inputs["g"], np.float32),
        np.asarray(inputs["b_fgl"], np.float32),
        np.asarray(inputs["W_fc"], np.float32),
        np.asarray(inputs["b_fc"], np.float32),
    )

    x_sh = np.ascontiguousarray(x).reshape(NCORES, NCH, 128, FREE)
    in_maps = [
        {"x": x_sh[i], "walm": walm, "warm": warm, "ccbt": ccbt}
        for i in range(NCORES)
    ]
    res = run_bass_kernel_spmd(nc, in_maps, list(range(NCORES)), trace=trace)
    y = np.concatenate([res.results[i]["y"] for i in range(NCORES)], axis=0)
    return y, res.exec_time_ns


def kernel(**inputs) -> np.ndarray:
    y, _ = _run(inputs, trace=False)
    return y
